# revision 5
# baseline (speedup 1.0000x reference)
"""PCEN kernel for Trainium2, SPMD across 8 NeuronCores.

Computes, for data [1, F=1024, T=16384] f32:
    M_t   = 0.5*M_{t-1} + 0.5*x_t          (EMA along T, per freq bin)
    Mepow = (M + 1e-6) ** alpha
    out   = (x / Mepow + delta) ** r - delta ** r     (r == 0.5)

Sharding: F across the 8 cores -> per-core shard [128, 16384], freq on
SBUF partitions, time on the free dimension.  Zero communication.

Per-core pipeline:
  phase A (ACT table set natural_log_exp_and_others):
    DMA x chunk -> DVE tensor_tensor_scan  state=(x+state)*0.5  (exact EMA)
    -> ACT Ln(M + eps) -> ACT Exp(-alpha * ln) = gain
    -> DVE t = x * gain, staged into a full-width SBUF buffer
  phase B (ACT table set sqrt_and_others):
    ACT Sqrt(t + delta) -> DVE (- delta**r) -> DMA out
The two-phase split keeps ACT at exactly two table loads (~2.7us each);
interleaving sqrt with ln/exp would thrash the spline table RAMs.
"""

from contextlib import ExitStack

import numpy as np

import concourse.tile as tile
from concourse import bacc, mybir
from concourse.bass_utils import run_bass_kernel_spmd

F_FULL = 1024
F_SHARD = 128
T = 16384
N_CORES = 8
EPS = 1e-6

CHUNK_A = 2048  # phase A chunk (scan / ln / exp / mul)
CHUNK_B = 2048  # phase B chunk (sqrt / sub / dma out)

_cache: dict = {}


def build(alpha: float, r: float, delta: float):
    assert abs(r - 0.5) < 1e-6, "kernel hardcodes r=0.5 (sqrt epilogue)"
    delta_r = float(np.float32(delta) ** np.float32(r))

    na = T // CHUNK_A
    nb = T // CHUNK_B

    nc = bacc.Bacc(
        "TRN2", target_bir_lowering=False, debug=False, num_devices=N_CORES
    )
    x_d = nc.dram_tensor(
        "data", [F_SHARD, T], mybir.dt.float32, kind="ExternalInput"
    ).ap()
    o_d = nc.dram_tensor(
        "out", [F_SHARD, T], mybir.dt.float32, kind="ExternalOutput"
    ).ap()

    f32 = mybir.dt.float32
    with tile.TileContext(nc) as tc, ExitStack() as ctx:
        constp = ctx.enter_context(tc.tile_pool(name="const", bufs=1))
        tfullp = ctx.enter_context(tc.tile_pool(name="tfull", bufs=1))
        xp = ctx.enter_context(tc.tile_pool(name="x", bufs=3))
        mp = ctx.enter_context(tc.tile_pool(name="m", bufs=2))
        lp = ctx.enter_context(tc.tile_pool(name="l", bufs=2))
        gp = ctx.enter_context(tc.tile_pool(name="g", bufs=2))
        up = ctx.enter_context(tc.tile_pool(name="u", bufs=2))
        op_ = ctx.enter_context(tc.tile_pool(name="o", bufs=2))

        half = constp.tile([F_SHARD, CHUNK_A], f32)
        nc.gpsimd.memset(half[:], 0.5)
        eps_b = constp.tile([F_SHARD, 1], f32, tag="epsb")
        nc.gpsimd.memset(eps_b[:], EPS)
        delta_b = constp.tile([F_SHARD, 1], f32, tag="deltab")
        nc.gpsimd.memset(delta_b[:], float(delta))

        t_full = tfullp.tile([F_SHARD, T], f32)

        m_prev = None
        last_exp = None
        for i in range(na):
            sl = slice(i * CHUNK_A, (i + 1) * CHUNK_A)
            x = xp.tile([F_SHARD, CHUNK_A], f32)
            nc.sync.dma_start(x[:], x_d[:, sl])
            m = mp.tile([F_SHARD, CHUNK_A], f32)
            init = 0.0 if m_prev is None else m_prev[:, CHUNK_A - 1 : CHUNK_A]
            nc.vector.tensor_tensor_scan(
                m[:],
                x[:],
                half[:],
                init,
                op0=mybir.AluOpType.add,
                op1=mybir.AluOpType.mult,
            )
            l = lp.tile([F_SHARD, CHUNK_A], f32)
            nc.scalar.activation(
                l[:], m[:], mybir.ActivationFunctionType.Ln, bias=eps_b[:]
            )
            g = gp.tile([F_SHARD, CHUNK_A], f32)
            last_exp = nc.scalar.activation(
                g[:], l[:], mybir.ActivationFunctionType.Exp, scale=-float(alpha)
            )
            nc.vector.tensor_mul(t_full[:, sl], x[:], g[:])
            m_prev = m

        for j in range(nb):
            sl = slice(j * CHUNK_B, (j + 1) * CHUNK_B)
            u = up.tile([F_SHARD, CHUNK_B], f32)
            s = nc.scalar.activation(
                u[:],
                t_full[:, sl],
                mybir.ActivationFunctionType.Sqrt,
                bias=delta_b[:],
            )
            # Keep every sqrt after the last exp on ACT so walrus emits only
            # one table-set switch (thrash costs ~2.7us per switch).
            tile.add_dep_helper(
                s.ins, last_exp.ins, sync=False, reason="act table phase order"
            )
            o = op_.tile([F_SHARD, CHUNK_B], f32)
            nc.vector.tensor_scalar_sub(o[:], u[:], delta_r)
            nc.sync.dma_start(o_d[:, sl], o[:])

    nc.compile()
    return nc


def _get_nc(alpha: float, r: float, delta: float):
    key = (alpha, r, delta)
    if key not in _cache:
        _cache[key] = build(alpha, r, delta)
    return _cache[key]


def make_in_maps(data: np.ndarray):
    x = np.ascontiguousarray(np.asarray(data, dtype=np.float32)[0])
    return [
        {"data": np.ascontiguousarray(x[k * F_SHARD : (k + 1) * F_SHARD])}
        for k in range(N_CORES)
    ]


def kernel(data, alpha, r, delta):
    a = float(np.asarray(alpha))
    rr = float(np.asarray(r))
    d = float(np.asarray(delta))
    nc = _get_nc(a, rr, d)
    in_maps = make_in_maps(data)
    res = run_bass_kernel_spmd(nc, in_maps, core_ids=list(range(N_CORES))).results
    out = np.concatenate([res[k]["out"] for k in range(N_CORES)], axis=0)
    return out[None].astype(np.float32, copy=False)


# revision 24
# speedup vs baseline: 24735.8468x; 24735.8468x over previous
"""PCEN kernel for Trainium2, SPMD across 8 NeuronCores.

Computes, for data [1, F=1024, T=16384] f32:
    M_t   = 0.5*M_{t-1} + 0.5*x_t          (EMA along T, per freq bin)
    Mepow = (M + 1e-6) ** alpha
    out   = (x / Mepow + delta) ** r - delta ** r     (r == 0.5)

Sharding: F across the 8 cores -> per-core shard [128, 16384], freq on
SBUF partitions, time on the free dimension.  Zero communication.

Per-core schedule (engine-balanced, ACT table-set aware):
  phase A (ACT set natural_log_exp_and_others), per chunk:
    DMA x chunk into full-width x buffer
    DVE tensor_tensor_scan  state=(x+state)*0.5   (exact EMA) -> m
    ACT Ln(m + eps) -> g;  ACT Exp(-alpha * g) -> g   (gain, in place)
    DVE g *= x   (pcen numerator x/(M+eps)^alpha, in place)
  phase B (ACT set sqrt_and_others), per chunk:
    ACT g = Sqrt(g + delta) (in place); DVE g -= delta**r; DMA out g
Phase split keeps ACT at exactly two spline-table loads (a switch costs
~2.7us and the sets {ln,exp} / {sqrt} are disjoint).

n_iters > 1 repeats the computation back-to-back in one NEFF (bench only).
"""

from contextlib import ExitStack

import numpy as np

import concourse.tile as tile
from concourse import bacc, mybir
from concourse.bass_utils import run_bass_kernel_spmd

F_FULL = 1024
F_SHARD = 128
T = 16384
N_CORES = 8
EPS = 1e-6

# phase A chunks: small lead-in so ACT starts early, wide middle to amortize
# per-instruction overhead (352 ACT cycles, 151 DVE cycles per chunk), small
# tail so the last dma->scan->ln->exp serial chain is short.
CHUNKS_A = [512, 512, 1024, 2048, 2048, 4096, 2048, 2048, 1024, 512, 512]
# phase B chunks: small head so out-DMA starts early, small tail so the
# last sqrt->sub->dma is short.
CHUNKS_B = [512, 512, 1024] + [2048] * 6 + [1024, 512, 512]
assert sum(CHUNKS_A) == T and sum(CHUNKS_B) == T

_cache: dict = {}


def build(alpha: float, r: float, delta: float, n_iters: int = 1):
    assert abs(r - 0.5) < 1e-6, "kernel hardcodes r=0.5 (sqrt epilogue)"
    delta_r = float(np.float32(delta) ** np.float32(r))

    nc = bacc.Bacc(
        "TRN2", target_bir_lowering=False, debug=False, num_devices=N_CORES
    )
    x_d = nc.dram_tensor(
        "data", [F_SHARD, T], mybir.dt.float32, kind="ExternalInput"
    ).ap()
    o_d = nc.dram_tensor(
        "out", [F_SHARD, T], mybir.dt.float32, kind="ExternalOutput"
    ).ap()

    f32 = mybir.dt.float32
    cmax = max(CHUNKS_A)
    with tile.TileContext(nc) as tc, ExitStack() as ctx:
        constp = ctx.enter_context(tc.tile_pool(name="const", bufs=1))
        xfullp = ctx.enter_context(tc.tile_pool(name="xfull", bufs=1))
        gfullp = ctx.enter_context(tc.tile_pool(name="gfull", bufs=1))
        mp = ctx.enter_context(tc.tile_pool(name="m", bufs=2))

        # `half` feeds every scan; a single wide memset would gate scan_0 by
        # ~4us.  Set the head (enough for the small lead-in chunks) on DVE
        # (fast, idle at t=0) and the wide rest on the otherwise-idle gpsimd.
        half = constp.tile([F_SHARD, cmax], f32)
        head = CHUNKS_A[0]
        nc.vector.memset(half[:, :head], 0.5)
        nc.gpsimd.memset(half[:, head:], 0.5)
        eps_b = constp.tile([F_SHARD, 1], f32, tag="epsb")
        nc.vector.memset(eps_b[:], EPS)
        delta_b = constp.tile([F_SHARD, 1], f32, tag="deltab")
        nc.vector.memset(delta_b[:], float(delta))

        x_full = xfullp.tile([F_SHARD, T], f32)
        g_full = gfullp.tile([F_SHARD, T], f32)

        # natural_log_exp_and_others covers both Ln and Exp.  Without this
        # pre-placed load the compiler picks a different set per function
        # (natural_log for Ln, exp_and_others for Exp) and reloads the
        # spline tables on every Ln<->Exp switch: 17 x 1.28us on HW.
        LNEXP_SET = 6  # index in act_info.json act_func_sets
        last_sqrt = None
        for it in range(n_iters):
            lnexp_load = nc.scalar.add_instruction(
                mybir.InstLoadActFuncSet(
                    name=nc.get_next_instruction_name(),
                    act_func_set_id=LNEXP_SET,
                    ins=[],
                    outs=[],
                )
            )
            if last_sqrt is not None:
                tile.add_dep_helper(
                    lnexp_load.ins, last_sqrt.ins, sync=False, reason="iter order"
                )
            m_prev = None
            last_exp = None
            pos = 0
            a_slices = []
            for c in CHUNKS_A:
                sl = slice(pos, pos + c)
                a_slices.append(sl)
                nc.sync.dma_start(x_full[:, sl], x_d[:, sl])
                m = mp.tile([F_SHARD, cmax], f32)
                init = 0.0 if m_prev is None else m_prev
                nc.vector.tensor_tensor_scan(
                    m[:, :c],
                    x_full[:, sl],
                    half[:, :c],
                    init,
                    op0=mybir.AluOpType.add,
                    op1=mybir.AluOpType.mult,
                )
                ln_i = nc.scalar.activation(
                    g_full[:, sl],
                    m[:, :c],
                    mybir.ActivationFunctionType.Ln,
                    bias=eps_b[:],
                )
                if last_sqrt is not None:
                    # bench mode: iteration k+1's ln/exp stay after iteration
                    # k's sqrts so per-iteration table-load behaviour matches
                    # the single-shot kernel.
                    tile.add_dep_helper(
                        ln_i.ins, last_sqrt.ins, sync=False, reason="iter order"
                    )
                last_exp = nc.scalar.activation(
                    g_full[:, sl],
                    g_full[:, sl],
                    mybir.ActivationFunctionType.Exp,
                    scale=-float(alpha),
                )
                m_prev = m[:, c - 1 : c]
                pos += c
            # muls emitted after every scan so scans win DVE priority; each
            # mul only needs its own exp, so they fill DVE gaps late in A.
            for sl in a_slices:
                nc.vector.tensor_mul(g_full[:, sl], x_full[:, sl], g_full[:, sl])

            pos = 0
            for c in CHUNKS_B:
                sl = slice(pos, pos + c)
                s = nc.scalar.activation(
                    g_full[:, sl],
                    g_full[:, sl],
                    mybir.ActivationFunctionType.Sqrt,
                    bias=delta_b[:],
                )
                # keep every sqrt after the last exp on ACT: one table switch
                tile.add_dep_helper(
                    s.ins, last_exp.ins, sync=False, reason="act table phase order"
                )
                nc.vector.tensor_scalar_sub(g_full[:, sl], g_full[:, sl], delta_r)
                nc.sync.dma_start(o_d[:, sl], g_full[:, sl])
                last_sqrt = s
                pos += c

    nc.compile()
    return nc


def _get_nc(alpha: float, r: float, delta: float):
    key = (alpha, r, delta)
    if key not in _cache:
        _cache[key] = build(alpha, r, delta)
    return _cache[key]


def make_in_maps(data: np.ndarray):
    x = np.ascontiguousarray(np.asarray(data, dtype=np.float32)[0])
    return [
        {"data": np.ascontiguousarray(x[k * F_SHARD : (k + 1) * F_SHARD])}
        for k in range(N_CORES)
    ]


def kernel(data, alpha, r, delta):
    a = float(np.asarray(alpha))
    rr = float(np.asarray(r))
    d = float(np.asarray(delta))
    nc = _get_nc(a, rr, d)
    in_maps = make_in_maps(data)
    res = run_bass_kernel_spmd(nc, in_maps, core_ids=list(range(N_CORES))).results
    out = np.concatenate([res[k]["out"] for k in range(N_CORES)], axis=0)
    return out[None].astype(np.float32, copy=False)
